# revision 2
# baseline (speedup 1.0000x reference)
"""3-layer GCN (PyG GCNConv semantics) on 8 Trainium2 NeuronCores — v2.

Strategy (graph/data parallel over nodes):
  - Nodes assigned to 8 cores x 196 tiles of 128 slots (serpentine by
    in-degree); edges (incl. self-loops) partitioned by destination tile,
    K chunks of 128 lanes per tile.
  - Aggregate-first per layer: per-chunk indirect-DMA gathers (128 rows
    per instruction; multi-index gathers only work for tiny tables on this
    HW), one-hot scatter matrices built on DVE (bf16), TensorE matmuls
    accumulate aggT [feature, dst] in PSUM.
  - Transform without transposes: out[dst, 384] = sum_k aggT_k^T @ W_k
    directly row-major in PSUM, ReLU on ACT -> bf16 row tiles.
  - Cross-core exchange via AllGather (8 cores = 4 devices x 2 cores;
    Shared-output dedups the pair). Layer 3 transform-first: z = H3 @ W3
    (2-wide) computed per tile during layer 2 (3 PE transposes + 3 tiny
    matmuls), so the second exchange is only ~100 KB/core; L3 aggregation
    gathers 4-byte rows and scatter-matmuls into [dst, 2] + b3.

kernel(**inputs) takes FULL unsharded inputs, returns FULL [200000, 2] f32.
"""

import time
from contextlib import ExitStack

import numpy as np

import concourse.bass as bass
import concourse.mybir as mybir
import concourse.tile as tile
from concourse import bacc
from concourse import bass_utils
from concourse._compat import axon_active
from concourse.bass import IndirectOffsetOnAxis

P = 128
F32 = mybir.dt.float32
BF16 = mybir.dt.bfloat16
I32 = mybir.dt.int32

N_NODES = 200000
F_IN = 165
HIDDEN = 384
F_OUT = 2
N_CORES = 8
TILES_PER_CORE = 196  # 8 * 196 * 128 = 200704 >= 200000
G = 7   # tiles per gather/scatter batch in layers 1-2 (196 = 28 * 7)
GZ = 49  # tiles per gather batch in layer 3

LAST_RESULTS = None


def _ceil_div(a, b):
    return (a + b - 1) // b


# --------------------------------------------------------------------------
# host-side preprocessing (topology only)
# --------------------------------------------------------------------------

def _preprocess(edge_index, n_nodes, n_cores, tiles_per_core):
    n_bins = n_cores * tiles_per_core
    n_pad = n_bins * P
    assert n_pad >= n_nodes

    row = np.asarray(edge_index[0], dtype=np.int64)
    col = np.asarray(edge_index[1], dtype=np.int64)
    loops = np.arange(n_nodes, dtype=np.int64)
    row_all = np.concatenate([row, loops])
    col_all = np.concatenate([col, loops])
    deg = np.bincount(col_all, minlength=n_nodes).astype(np.float64)  # >= 1
    dinv = 1.0 / np.sqrt(deg)
    nrm_all = (dinv[row_all] * dinv[col_all]).astype(np.float32)

    # node -> (bin, slot): serpentine over bins in descending-degree order
    d = np.zeros(n_pad, np.int64)
    d[:n_nodes] = deg.astype(np.int64)
    order = np.argsort(-d, kind="stable")
    rows_idx = np.arange(n_pad) // n_bins
    pos = np.arange(n_pad) % n_bins
    bins_of_rank = np.where(rows_idx % 2 == 0, pos, n_bins - 1 - pos)
    bin_of_node = np.empty(n_pad, np.int64)
    slot_of_node = np.empty(n_pad, np.int64)
    bin_of_node[order] = bins_of_rank
    slot_of_node[order] = rows_idx
    perm = bin_of_node * P + slot_of_node

    load = np.bincount(bin_of_node[:n_nodes], weights=deg, minlength=n_bins)
    K = int(np.ceil(load.max() / P))

    # edge tables: chunk = K per tile, 128 lanes per chunk
    e_src_dev = perm[row_all]
    e_dst_dev = perm[col_all]
    e_bin = e_dst_dev // P
    e_slot = e_dst_dev % P
    eo = np.argsort(e_bin, kind="stable")
    e_bin_s = e_bin[eo]
    cnt = np.bincount(e_bin_s, minlength=n_bins)
    starts = np.concatenate([[0], np.cumsum(cnt)[:-1]])
    within = np.arange(len(e_bin_s)) - starts[e_bin_s]
    lane = within % P
    chunk_global = e_bin_s * K + within // P

    n_ch = n_bins * K
    src_t = np.zeros((n_ch, P), np.int32)
    dst_t = np.zeros((n_ch, P), np.float32)
    nrm_t = np.zeros((n_ch, P), np.float32)
    src_t[chunk_global, lane] = e_src_dev[eo].astype(np.int32)
    dst_t[chunk_global, lane] = e_slot[eo].astype(np.float32)
    nrm_t[chunk_global, lane] = nrm_all[eo]

    TK = tiles_per_core * K
    src = src_t.reshape(n_cores, TK, P).transpose(0, 2, 1).copy()
    dstf = dst_t.reshape(n_cores, TK, P).transpose(0, 2, 1).copy()
    nrm = nrm_t.reshape(n_cores, TK, P).transpose(0, 2, 1).copy()
    return dict(perm=perm, K=K, src=src, dstf=dstf, nrm=nrm, n_pad=n_pad)


def _pack_w(W, f_out):
    f_in = W.shape[0]
    kc = _ceil_div(f_in, P)
    Wp = np.zeros((kc * P, f_out), np.float32)
    Wp[:f_in] = np.asarray(W, np.float32)
    return Wp.reshape(kc, P, f_out).transpose(1, 0, 2).reshape(P, kc * f_out).copy()


# --------------------------------------------------------------------------
# device program
# --------------------------------------------------------------------------

def _build_gcn(tc, ins, out_ap, cfg, dbg_aps=None):
    nc = tc.nc
    n_cores = cfg["n_cores"]
    T, K = cfg["T"], cfg["K"]
    F1, H, O = cfg["F1"], cfg["H"], cfg["O"]
    use_b1, use_b2 = cfg["use_b1"], cfg["use_b2"]
    n_pad = n_cores * T * P
    kc1 = _ceil_div(F1, P)
    kc2 = _ceil_div(H, P)
    rg = [list(range(n_cores))]

    ctx = ExitStack()
    with ctx:
        const = ctx.enter_context(tc.tile_pool(name="const", bufs=1))
        dram = ctx.enter_context(tc.tile_pool(name="dram", bufs=1, space="DRAM"))
        work = ctx.enter_context(tc.tile_pool(name="work", bufs=2))
        psum = ctx.enter_context(tc.tile_pool(name="psum", bufs=2, space="PSUM"))

        def load_const(name, shape, dtype=F32):
            t = const.tile(list(shape), dtype, name=name)
            nc.sync.dma_start(out=t[:], in_=ins[name][:])
            return t

        iota_bf = load_const("iota", [P, P], BF16)
        ident_bf = load_const("ident", [P, P], BF16)
        w1_sb = load_const("w1", [P, kc1 * H], BF16)
        w2_sb = load_const("w2", [P, kc2 * H], BF16)
        w3_sb = load_const("w3", [P, kc2 * O], BF16)
        b3row_sb = load_const("b3row", [P, O])
        src_sb = load_const("src", [P, T * K], I32)
        dstf_sb = load_const("dstf", [P, T * K])
        nrm_sb = load_const("nrm", [P, T * K])
        if use_b1 or use_b2:
            ones_sb = load_const("ones", [1, P], BF16)
        if use_b1:
            b1row_sb = load_const("b1row", [1, H], BF16)
        if use_b2:
            b2row_sb = load_const("b2row", [1, H], BF16)

        outbuf = const.tile([P, T * O], F32, name="outbuf")

        x2s = dram.tile([T * P, H], BF16, name="x2s")
        x2f = dram.tile([n_pad, H], BF16, name="x2f", addr_space="Shared")
        zs = dram.tile([T * P, O], BF16, name="zs")
        zf = dram.tile([n_pad, O], BF16, name="zf", addr_space="Shared")

        def agg_tile(x_src_ap, F, kc, t):
            """aggT[f, dst] = sum_c gathered_c[:, f]^T @ onehot_c"""
            hgs, mhs = [], []
            for c in range(K):
                ch = t * K + c
                hg = work.tile([P, H], BF16, name="hg", tag="hg", bufs=6)
                nc.gpsimd.indirect_dma_start(
                    out=hg[:, :F],
                    out_offset=None,
                    in_=x_src_ap,
                    in_offset=IndirectOffsetOnAxis(
                        ap=src_sb[:, ch:ch + 1], axis=0),
                )
                mh = work.tile([P, P], BF16, name="mh", tag="mh", bufs=6)
                nc.vector.tensor_scalar(
                    out=mh[:],
                    in0=iota_bf[:],
                    scalar1=dstf_sb[:, ch:ch + 1],
                    scalar2=nrm_sb[:, ch:ch + 1],
                    op0=mybir.AluOpType.is_equal,
                    op1=mybir.AluOpType.mult,
                )
                hgs.append(hg)
                mhs.append(mh)
            aggT = work.tile([P, kc2 * P], BF16, name="aggT", tag="aggT", bufs=3)
            for f in range(kc):
                fw = min(P, F - f * P)
                pa = psum.tile([P, P], F32, name="pa", tag="pa", bufs=2)
                for c in range(K):
                    nc.tensor.matmul(
                        out=pa[:fw, :],
                        lhsT=hgs[c][:, f * P:f * P + fw],
                        rhs=mhs[c][:],
                        start=(c == 0),
                        stop=(c == K - 1),
                    )
                nc.scalar.copy(out=aggT[:fw, f * P:(f + 1) * P], in_=pa[:fw, :])
            return aggT

        def transform_tile(aggT, kc_in, F_in, w_sb, brow):
            """pt[dst, j] = sum_k aggT_k^T @ W_k (+ ones^T @ brow)"""
            pt = psum.tile([P, H], F32, name="pt", tag="pt", bufs=2)
            for k in range(kc_in):
                kw = min(P, F_in - k * P)
                nc.tensor.matmul(
                    out=pt[:, :H],
                    lhsT=aggT[:kw, k * P:k * P + P],
                    rhs=w_sb[:kw, k * H:k * H + H],
                    start=(k == 0),
                    stop=(k == kc_in - 1 and brow is None),
                )
            if brow is not None:
                nc.tensor.matmul(
                    out=pt[:, :H],
                    lhsT=ones_sb[:1, :P],
                    rhs=brow[:1, :H],
                    start=False,
                    stop=True,
                )
            return pt

        nb = T // G

        # ---------------- layer 1 ----------------
        for b in range(nb):
            xsb = work.tile([P, G * H], BF16, name="xsb", tag="xsb", bufs=2)
            for g in range(G):
                t = b * G + g
                aggT = agg_tile(ins["x"][:], F1, kc1, t)
                if dbg_aps is not None and t == 0:
                    nc.sync.dma_start(out=dbg_aps["dbg_agg"], in_=aggT[:])
                pt = transform_tile(aggT, kc1, F1, w1_sb,
                                    b1row_sb if use_b1 else None)
                nc.scalar.activation(
                    out=xsb[:, g * H:(g + 1) * H],
                    in_=pt[:, :H],
                    func=mybir.ActivationFunctionType.Relu,
                )
                nc.sync.dma_start(out=x2s[t * P:(t + 1) * P, :],
                                  in_=xsb[:, g * H:(g + 1) * H])
                if dbg_aps is not None:
                    nc.sync.dma_start(out=dbg_aps["dbg_x2"][t * P:(t + 1) * P, :],
                                      in_=xsb[:, g * H:(g + 1) * H])

        nc.gpsimd.collective_compute(
            "AllGather", mybir.AluOpType.bypass, replica_groups=rg,
            ins=[x2s.opt()], outs=[x2f.opt()],
        )

        # ---------------- layer 2 (+ z = relu-out @ W3) ----------------
        for b in range(nb):
            zsb = work.tile([P, G * O], BF16, name="zsb", tag="zsb", bufs=2)
            for g in range(G):
                t = b * G + g
                aggT = agg_tile(x2f[:], H, kc2, t)
                pt = transform_tile(aggT, kc2, H, w2_sb,
                                    b2row_sb if use_b2 else None)
                x3row = work.tile([P, H], BF16, name="x3r", tag="x3r", bufs=3)
                nc.scalar.activation(
                    out=x3row[:],
                    in_=pt[:, :H],
                    func=mybir.ActivationFunctionType.Relu,
                )
                xTs = []
                for k in range(kc2):
                    ptp = psum.tile([P, P], BF16, name="ptp", tag="ptp", bufs=2)
                    nc.tensor.transpose(
                        out=ptp[:],
                        in_=x3row[:, k * P:(k + 1) * P],
                        identity=ident_bf[:],
                    )
                    xT = work.tile([P, P], BF16, name="xT", tag="xT", bufs=4)
                    nc.vector.tensor_copy(out=xT[:], in_=ptp[:])
                    xTs.append(xT)
                pz = psum.tile([P, O], F32, name="pz", tag="pz", bufs=2)
                for k in range(kc2):
                    nc.tensor.matmul(
                        out=pz[:, :O],
                        lhsT=xTs[k][:],
                        rhs=w3_sb[:, k * O:(k + 1) * O],
                        start=(k == 0),
                        stop=(k == kc2 - 1),
                    )
                nc.scalar.copy(out=zsb[:, g * O:(g + 1) * O], in_=pz[:, :O])
                nc.sync.dma_start(out=zs[t * P:(t + 1) * P, :],
                                  in_=zsb[:, g * O:(g + 1) * O])
                if dbg_aps is not None:
                    nc.sync.dma_start(out=dbg_aps["dbg_z"][t * P:(t + 1) * P, :],
                                      in_=zsb[:, g * O:(g + 1) * O])

        nc.gpsimd.collective_compute(
            "AllGather", mybir.AluOpType.bypass, replica_groups=rg,
            ins=[zs.opt()], outs=[zf.opt()],
        )

        # ---------------- layer 3: aggregate z ----------------
        for t in range(T):
            po = psum.tile([P, O], F32, name="po", tag="pz", bufs=2)
            for c in range(K):
                ch = t * K + c
                hgz = work.tile([P, O], BF16, name="hgz", tag="hgz", bufs=6)
                nc.gpsimd.indirect_dma_start(
                    out=hgz[:],
                    out_offset=None,
                    in_=zf[:],
                    in_offset=IndirectOffsetOnAxis(
                        ap=src_sb[:, ch:ch + 1], axis=0),
                )
                mh = work.tile([P, P], BF16, name="mh", tag="mh", bufs=6)
                nc.vector.tensor_scalar(
                    out=mh[:],
                    in0=iota_bf[:],
                    scalar1=dstf_sb[:, ch:ch + 1],
                    scalar2=nrm_sb[:, ch:ch + 1],
                    op0=mybir.AluOpType.is_equal,
                    op1=mybir.AluOpType.mult,
                )
                nc.tensor.matmul(
                    out=po[:, :O],
                    lhsT=mh[:],
                    rhs=hgz[:],
                    start=(c == 0),
                    stop=(c == K - 1),
                )
            nc.vector.tensor_tensor(
                out=outbuf[:, t * O:(t + 1) * O],
                in0=po[:, :O],
                in1=b3row_sb[:, :O],
                op=mybir.AluOpType.add,
            )
        nc.sync.dma_start(out=out_ap, in_=outbuf[:])


# --------------------------------------------------------------------------
# execution (axon / PJRT path with device-resident timing)
# --------------------------------------------------------------------------

EXEC_NS = None


def _run_pjrt_timed(nc, in_maps, n_cores, time_iters=0):
    global EXEC_NS
    import jax
    import jax.numpy as jnp  # noqa: F401
    from jax.experimental.shard_map import shard_map
    from jax.sharding import Mesh, NamedSharding, PartitionSpec

    from concourse import bass2jax as b2j

    b2j.install_neuronx_cc_hook()

    partition_name = (nc.partition_id_tensor.name
                      if nc.partition_id_tensor else None)
    in_names, out_names, out_avals, zero_outs = [], [], [], []
    for alloc in nc.m.functions[0].allocations:
        if not isinstance(alloc, mybir.MemoryLocationSet):
            continue
        name = alloc.memorylocations[0].name
        if alloc.kind == "ExternalInput":
            if name != partition_name:
                in_names.append(name)
        elif alloc.kind == "ExternalOutput":
            out_names.append(name)
            shape = tuple(alloc.tensor_shape)
            dtype = mybir.dt.np(alloc.dtype)
            out_avals.append(jax.core.ShapedArray(shape, dtype))
            zero_outs.append(np.zeros(shape, dtype))
    n_params = len(in_names)
    all_in_names = list(in_names) + list(out_names)
    if partition_name is not None:
        all_in_names.append(partition_name)
    all_in_names = tuple(all_in_names)

    def _body(*args):
        operands = list(args)
        if partition_name is not None:
            operands.append(b2j.partition_id_tensor())
        outs = b2j._bass_exec_p.bind(
            *operands,
            out_avals=tuple(out_avals),
            in_names=all_in_names,
            out_names=tuple(out_names),
            lowering_input_output_aliases=(),
            sim_require_finite=True,
            sim_require_nnan=True,
            nc=nc,
        )
        return tuple(outs)

    devices = jax.devices()[:n_cores]
    assert len(devices) == n_cores
    mesh = Mesh(np.asarray(devices), ("core",))
    spec = PartitionSpec("core")
    n_all = n_params + len(zero_outs)
    jitted = jax.jit(shard_map(
        _body, mesh=mesh, in_specs=(spec,) * n_all,
        out_specs=(spec,) * len(out_names), check_rep=False))

    sharding = NamedSharding(mesh, spec)
    g_in = [
        jax.device_put(
            np.concatenate([np.asarray(in_maps[c][nm]) for c in range(n_cores)],
                           axis=0), sharding)
        for nm in in_names
    ]
    g_zero = [
        jax.device_put(np.concatenate([z] * n_cores, axis=0), sharding)
        for z in zero_outs
    ]

    out_arrs = jitted(*g_in, *g_zero)
    jax.block_until_ready(out_arrs)
    results = [
        {nm: np.asarray(out_arrs[i]).reshape(n_cores, *out_avals[i].shape)[c]
         for i, nm in enumerate(out_names)}
        for c in range(n_cores)
    ]

    if time_iters > 0:
        triv = jax.jit(shard_map(
            lambda a: (a + 1.0,), mesh=mesh, in_specs=(spec,),
            out_specs=(spec,), check_rep=False))
        tiny = jax.device_put(np.zeros((n_cores * 8, 8), np.float32), sharding)
        jax.block_until_ready(triv(tiny))
        walls, base = [], []
        for _ in range(time_iters):
            t0 = time.perf_counter()
            o = jitted(*g_in, *g_zero)
            jax.block_until_ready(o)
            walls.append(time.perf_counter() - t0)
            t0 = time.perf_counter()
            o = triv(tiny)
            jax.block_until_ready(o)
            base.append(time.perf_counter() - t0)
        walls = np.array(walls)
        base = np.array(base)
        diffs = walls - base
        EXEC_NS = int((np.min(walls) - np.min(base)) * 1e9)
        print(f"[timing] kernel min {np.min(walls)*1e3:.3f} "
              f"med {np.median(walls)*1e3:.3f} ms | base min "
              f"{np.min(base)*1e3:.3f} med {np.median(base)*1e3:.3f} ms | "
              f"min-diff {EXEC_NS/1e3:.0f} us  med-diff "
              f"{np.median(diffs)*1e6:.0f} us")
    return results


# --------------------------------------------------------------------------
# top level
# --------------------------------------------------------------------------

def kernel(x, edge_index, W1, b1, W2, b2, W3, b3, _trace=False, _time_iters=0, _debug=False):
    global LAST_RESULTS
    x = np.asarray(x, np.float32)
    edge_index = np.asarray(edge_index)
    n_nodes = x.shape[0]
    assert n_nodes == N_NODES and x.shape[1] == F_IN

    pre = _preprocess(edge_index, n_nodes, N_CORES, TILES_PER_CORE)
    T, K = TILES_PER_CORE, pre["K"]
    n_pad = pre["n_pad"]
    use_b1 = bool(np.any(np.asarray(b1) != 0))
    use_b2 = bool(np.any(np.asarray(b2) != 0))
    cfg = dict(n_cores=N_CORES, T=T, K=K, F1=F_IN, H=HIDDEN, O=F_OUT,
               use_b1=use_b1, use_b2=use_b2)

    import ml_dtypes
    bf = ml_dtypes.bfloat16
    x_dev = np.zeros((n_pad, F_IN), bf)
    x_dev[pre["perm"][:n_nodes]] = x

    common = dict(
        x=x_dev,
        iota=np.tile(np.arange(P, dtype=np.float32), (P, 1)).astype(bf),
        ident=np.eye(P, dtype=np.float32).astype(bf),
        w1=_pack_w(W1, HIDDEN).astype(bf),
        w2=_pack_w(W2, HIDDEN).astype(bf),
        w3=_pack_w(W3, F_OUT).astype(bf),
        b3row=np.tile(np.asarray(b3, np.float32), (P, 1)).copy(),
    )
    if use_b1 or use_b2:
        common["ones"] = np.ones((1, P), bf)
    if use_b1:
        common["b1row"] = np.asarray(b1, np.float32).reshape(1, HIDDEN).astype(bf)
    if use_b2:
        common["b2row"] = np.asarray(b2, np.float32).reshape(1, HIDDEN).astype(bf)

    in_maps = []
    for c in range(N_CORES):
        m = dict(common)
        m["src"] = pre["src"][c]
        m["dstf"] = pre["dstf"][c]
        m["nrm"] = pre["nrm"][c]
        in_maps.append(m)

    nc = bacc.Bacc("TRN2", target_bir_lowering=False, debug=False,
                   enable_asserts=False, num_devices=N_CORES)
    ins_aps = {}
    for name, arr in in_maps[0].items():
        ins_aps[name] = nc.dram_tensor(
            name, list(arr.shape), mybir.dt.from_np(arr.dtype),
            kind="ExternalInput").ap()
    out_t = nc.dram_tensor("out", [P, T * F_OUT], F32, kind="ExternalOutput")
    dbg_aps = None
    if _debug:
        kc1 = _ceil_div(F_IN, P)
        kc2 = _ceil_div(HIDDEN, P)
        dbg_aps = {
            "dbg_agg": nc.dram_tensor("dbg_agg", [P, kc2 * P], BF16,
                                      kind="ExternalOutput").ap(),
            "dbg_x2": nc.dram_tensor("dbg_x2", [T * P, HIDDEN], BF16,
                                     kind="ExternalOutput").ap(),
            "dbg_z": nc.dram_tensor("dbg_z", [T * P, F_OUT], BF16,
                                    kind="ExternalOutput").ap(),
        }

    with tile.TileContext(nc) as tc:
        _build_gcn(tc, ins_aps, out_t.ap(), cfg, dbg_aps)
    nc.compile()

    if axon_active():
        results = _run_pjrt_timed(nc, in_maps, N_CORES, time_iters=_time_iters)
    else:
        res = bass_utils.run_bass_kernel_spmd(
            nc, in_maps, core_ids=list(range(N_CORES)), trace=_trace)
        LAST_RESULTS = res
        results = res.results

    out_dev = np.zeros((n_pad, F_OUT), np.float32)
    for c in range(N_CORES):
        o = results[c]["out"]  # [P, T*O]
        rows = o.reshape(P, T, F_OUT).transpose(1, 0, 2).reshape(T * P, F_OUT)
        out_dev[c * T * P:(c + 1) * T * P] = rows
    out_full = out_dev[pre["perm"][:n_nodes]].copy()
    if _debug:
        return out_full, results, pre, in_maps
    return out_full


# revision 3
# speedup vs baseline: 1.0155x; 1.0155x over previous
"""3-layer GCN (PyG GCNConv semantics) on 8 Trainium2 NeuronCores — v2.

Strategy (graph/data parallel over nodes):
  - Nodes assigned to 8 cores x 196 tiles of 128 slots (serpentine by
    in-degree); edges (incl. self-loops) partitioned by destination tile,
    K chunks of 128 lanes per tile.
  - Aggregate-first per layer: per-chunk indirect-DMA gathers (128 rows
    per instruction; multi-index gathers only work for tiny tables on this
    HW), one-hot scatter matrices built on DVE (bf16), TensorE matmuls
    accumulate aggT [feature, dst] in PSUM.
  - Transform without transposes: out[dst, 384] = sum_k aggT_k^T @ W_k
    directly row-major in PSUM, ReLU on ACT -> bf16 row tiles.
  - Cross-core exchange via AllGather (8 cores = 4 devices x 2 cores;
    Shared-output dedups the pair). Layer 3 transform-first: z = H3 @ W3
    (2-wide) computed per tile during layer 2 (3 PE transposes + 3 tiny
    matmuls), so the second exchange is only ~100 KB/core; L3 aggregation
    gathers 4-byte rows and scatter-matmuls into [dst, 2] + b3.

kernel(**inputs) takes FULL unsharded inputs, returns FULL [200000, 2] f32.
"""

import time
from contextlib import ExitStack

import numpy as np

import concourse.bass as bass
import concourse.mybir as mybir
import concourse.tile as tile
from concourse import bacc
from concourse import bass_utils
from concourse._compat import axon_active
from concourse.bass import IndirectOffsetOnAxis

P = 128
F32 = mybir.dt.float32
BF16 = mybir.dt.bfloat16
I32 = mybir.dt.int32

N_NODES = 200000
F_IN = 165
HIDDEN = 384
F_OUT = 2
N_CORES = 8
TILES_PER_CORE = 196  # 8 * 196 * 128 = 200704 >= 200000
G = 7   # tiles per gather/scatter batch in layers 1-2 (196 = 28 * 7)
GZ = 49  # tiles per gather batch in layer 3

LAST_RESULTS = None


def _ceil_div(a, b):
    return (a + b - 1) // b


# --------------------------------------------------------------------------
# host-side preprocessing (topology only)
# --------------------------------------------------------------------------

def _preprocess(edge_index, n_nodes, n_cores, tiles_per_core):
    n_bins = n_cores * tiles_per_core
    n_pad = n_bins * P
    assert n_pad >= n_nodes

    row_all = np.asarray(edge_index[0], dtype=np.int64)
    col_all = np.asarray(edge_index[1], dtype=np.int64)
    deg = np.bincount(col_all, minlength=n_nodes).astype(np.float64) + 1.0
    dinv = 1.0 / np.sqrt(deg)
    nrm_all = (dinv[row_all] * dinv[col_all]).astype(np.float32)

    # node -> (bin, slot): serpentine over bins in descending-degree order
    d = np.zeros(n_pad, np.int64)
    d[:n_nodes] = deg.astype(np.int64)
    order = np.argsort(-d, kind="stable")
    rows_idx = np.arange(n_pad) // n_bins
    pos = np.arange(n_pad) % n_bins
    bins_of_rank = np.where(rows_idx % 2 == 0, pos, n_bins - 1 - pos)
    bin_of_node = np.empty(n_pad, np.int64)
    slot_of_node = np.empty(n_pad, np.int64)
    bin_of_node[order] = bins_of_rank
    slot_of_node[order] = rows_idx
    perm = bin_of_node * P + slot_of_node

    rdeg = np.bincount(col_all, minlength=n_nodes).astype(np.float64)
    binload = np.bincount(bin_of_node[:n_nodes], weights=rdeg, minlength=n_bins)
    K = int(np.ceil(binload.max() / P))

    # edge tables: chunk = K per tile, 128 lanes per chunk
    e_src_dev = perm[row_all]
    e_dst_dev = perm[col_all]
    e_bin = e_dst_dev // P
    e_slot = e_dst_dev % P
    eo = np.argsort(e_bin, kind="stable")
    e_bin_s = e_bin[eo]
    cnt = np.bincount(e_bin_s, minlength=n_bins)
    starts = np.concatenate([[0], np.cumsum(cnt)[:-1]])
    within = np.arange(len(e_bin_s)) - starts[e_bin_s]
    lane = within % P
    chunk_global = e_bin_s * K + within // P

    n_ch = n_bins * K
    src_t = np.zeros((n_ch, P), np.int32)
    dst_t = np.zeros((n_ch, P), np.float32)
    nrm_t = np.zeros((n_ch, P), np.float32)
    src_t[chunk_global, lane] = e_src_dev[eo].astype(np.int32)
    dst_t[chunk_global, lane] = e_slot[eo].astype(np.float32)
    nrm_t[chunk_global, lane] = nrm_all[eo]

    TK = tiles_per_core * K
    src = src_t.reshape(n_cores, TK, P).transpose(0, 2, 1).copy()
    dstf = dst_t.reshape(n_cores, TK, P).transpose(0, 2, 1).copy()
    nrm = nrm_t.reshape(n_cores, TK, P).transpose(0, 2, 1).copy()

    node_of_row = np.full(n_pad, -1, np.int64)
    node_of_row[perm[:n_nodes]] = np.arange(n_nodes)
    dinv2_row = np.zeros(n_pad, np.float32)
    ok = node_of_row >= 0
    dinv2_row[ok] = (dinv[node_of_row[ok]] ** 2).astype(np.float32)
    selfnrm = dinv2_row.reshape(n_cores, tiles_per_core, P).transpose(0, 2, 1).copy()
    return dict(perm=perm, K=K, src=src, dstf=dstf, nrm=nrm, n_pad=n_pad,
                selfnrm=selfnrm)


def _pack_w(W, f_out):
    f_in = W.shape[0]
    kc = _ceil_div(f_in, P)
    Wp = np.zeros((kc * P, f_out), np.float32)
    Wp[:f_in] = np.asarray(W, np.float32)
    return Wp.reshape(kc, P, f_out).transpose(1, 0, 2).reshape(P, kc * f_out).copy()


# --------------------------------------------------------------------------
# device program
# --------------------------------------------------------------------------

def _build_gcn(tc, ins, out_ap, cfg, dbg_aps=None):
    nc = tc.nc
    n_cores = cfg["n_cores"]
    T, K = cfg["T"], cfg["K"]
    F1, H, O = cfg["F1"], cfg["H"], cfg["O"]
    use_b1, use_b2 = cfg["use_b1"], cfg["use_b2"]
    n_pad = n_cores * T * P
    kc1 = _ceil_div(F1, P)
    kc2 = _ceil_div(H, P)
    rg = [list(range(n_cores))]

    ctx = ExitStack()
    with ctx:
        const = ctx.enter_context(tc.tile_pool(name="const", bufs=1))
        dram = ctx.enter_context(tc.tile_pool(name="dram", bufs=1, space="DRAM"))
        work = ctx.enter_context(tc.tile_pool(name="work", bufs=2))
        psum = ctx.enter_context(tc.tile_pool(name="psum", bufs=2, space="PSUM"))

        def load_const(name, shape, dtype=F32):
            t = const.tile(list(shape), dtype, name=name)
            nc.sync.dma_start(out=t[:], in_=ins[name][:])
            return t

        iota_bf = load_const("iota", [P, P], BF16)
        ident_bf = load_const("ident", [P, P], BF16)
        w1_sb = load_const("w1", [P, kc1 * H], BF16)
        w2_sb = load_const("w2", [P, kc2 * H], BF16)
        w3_sb = load_const("w3", [P, kc2 * O], BF16)
        b3row_sb = load_const("b3row", [P, O])
        src_sb = load_const("src", [P, T * K], I32)
        dstf_sb = load_const("dstf", [P, T * K])
        nrm_sb = load_const("nrm", [P, T * K])
        selfdst_sb = load_const("selfdst", [P, 1])
        selfnrm_sb = load_const("selfnrm", [P, T])
        if use_b1 or use_b2:
            ones_sb = load_const("ones", [1, P], BF16)
        if use_b1:
            b1row_sb = load_const("b1row", [1, H], BF16)
        if use_b2:
            b2row_sb = load_const("b2row", [1, H], BF16)

        outbuf = const.tile([P, T * O], F32, name="outbuf")

        x2s = dram.tile([T * P, H], BF16, name="x2s")
        x2f = dram.tile([n_pad, H], BF16, name="x2f", addr_space="Shared")
        zs = dram.tile([T * P, O], BF16, name="zs")
        zf = dram.tile([n_pad, O], BF16, name="zf", addr_space="Shared")

        def agg_tile(x_src_ap, self_src_ap, F, kc, t):
            """aggT[f, dst] = sum_c gathered_c[:, f]^T @ onehot_c
            plus the self-loop chunk loaded with a plain contiguous DMA"""
            hgs, mhs = [], []
            for c in range(K):
                ch = t * K + c
                hg = work.tile([P, H], BF16, name="hg", tag="hg", bufs=8)
                nc.gpsimd.indirect_dma_start(
                    out=hg[:, :F],
                    out_offset=None,
                    in_=x_src_ap,
                    in_offset=IndirectOffsetOnAxis(
                        ap=src_sb[:, ch:ch + 1], axis=0),
                )
                mh = work.tile([P, P], BF16, name="mh", tag="mh", bufs=8)
                nc.vector.tensor_scalar(
                    out=mh[:],
                    in0=iota_bf[:],
                    scalar1=dstf_sb[:, ch:ch + 1],
                    scalar2=nrm_sb[:, ch:ch + 1],
                    op0=mybir.AluOpType.is_equal,
                    op1=mybir.AluOpType.mult,
                )
                hgs.append(hg)
                mhs.append(mh)
            hs = work.tile([P, H], BF16, name="hg", tag="hg", bufs=8)
            nc.sync.dma_start(out=hs[:, :F],
                              in_=self_src_ap[t * P:(t + 1) * P, :])
            ms = work.tile([P, P], BF16, name="mh", tag="mh", bufs=8)
            nc.vector.tensor_scalar(
                out=ms[:],
                in0=iota_bf[:],
                scalar1=selfdst_sb[:, 0:1],
                scalar2=selfnrm_sb[:, t:t + 1],
                op0=mybir.AluOpType.is_equal,
                op1=mybir.AluOpType.mult,
            )
            hgs.append(hs)
            mhs.append(ms)
            aggT = work.tile([P, kc2 * P], BF16, name="aggT", tag="aggT", bufs=3)
            for f in range(kc):
                fw = min(P, F - f * P)
                pa = psum.tile([P, P], F32, name="pa", tag="pa", bufs=2)
                for c in range(K + 1):
                    nc.tensor.matmul(
                        out=pa[:fw, :],
                        lhsT=hgs[c][:, f * P:f * P + fw],
                        rhs=mhs[c][:],
                        start=(c == 0),
                        stop=(c == K),
                    )
                nc.scalar.copy(out=aggT[:fw, f * P:(f + 1) * P], in_=pa[:fw, :])
            return aggT

        def transform_tile(aggT, kc_in, F_in, w_sb, brow):
            """pt[dst, j] = sum_k aggT_k^T @ W_k (+ ones^T @ brow)"""
            pt = psum.tile([P, H], F32, name="pt", tag="pt", bufs=2)
            for k in range(kc_in):
                kw = min(P, F_in - k * P)
                nc.tensor.matmul(
                    out=pt[:, :H],
                    lhsT=aggT[:kw, k * P:k * P + P],
                    rhs=w_sb[:kw, k * H:k * H + H],
                    start=(k == 0),
                    stop=(k == kc_in - 1 and brow is None),
                )
            if brow is not None:
                nc.tensor.matmul(
                    out=pt[:, :H],
                    lhsT=ones_sb[:1, :P],
                    rhs=brow[:1, :H],
                    start=False,
                    stop=True,
                )
            return pt

        nb = T // G

        # ---------------- layer 1 ----------------
        for b in range(nb):
            xsb = work.tile([P, G * H], BF16, name="xsb", tag="xsb", bufs=2)
            for g in range(G):
                t = b * G + g
                aggT = agg_tile(ins["x"][:], ins["xown"], F1, kc1, t)
                if dbg_aps is not None and t == 0:
                    nc.sync.dma_start(out=dbg_aps["dbg_agg"], in_=aggT[:])
                pt = transform_tile(aggT, kc1, F1, w1_sb,
                                    b1row_sb if use_b1 else None)
                nc.scalar.activation(
                    out=xsb[:, g * H:(g + 1) * H],
                    in_=pt[:, :H],
                    func=mybir.ActivationFunctionType.Relu,
                )
                nc.sync.dma_start(out=x2s[t * P:(t + 1) * P, :],
                                  in_=xsb[:, g * H:(g + 1) * H])
                if dbg_aps is not None:
                    nc.sync.dma_start(out=dbg_aps["dbg_x2"][t * P:(t + 1) * P, :],
                                      in_=xsb[:, g * H:(g + 1) * H])

        nc.gpsimd.collective_compute(
            "AllGather", mybir.AluOpType.bypass, replica_groups=rg,
            ins=[x2s.opt()], outs=[x2f.opt()],
        )

        # ---------------- layer 2 (+ z = relu-out @ W3) ----------------
        for b in range(nb):
            zsb = work.tile([P, G * O], BF16, name="zsb", tag="zsb", bufs=2)
            for g in range(G):
                t = b * G + g
                aggT = agg_tile(x2f[:], x2s, H, kc2, t)
                pt = transform_tile(aggT, kc2, H, w2_sb,
                                    b2row_sb if use_b2 else None)
                x3row = work.tile([P, H], BF16, name="x3r", tag="x3r", bufs=3)
                nc.scalar.activation(
                    out=x3row[:],
                    in_=pt[:, :H],
                    func=mybir.ActivationFunctionType.Relu,
                )
                xTs = []
                for k in range(kc2):
                    ptp = psum.tile([P, P], BF16, name="ptp", tag="ptp", bufs=2)
                    nc.tensor.transpose(
                        out=ptp[:],
                        in_=x3row[:, k * P:(k + 1) * P],
                        identity=ident_bf[:],
                    )
                    xT = work.tile([P, P], BF16, name="xT", tag="xT", bufs=4)
                    nc.vector.tensor_copy(out=xT[:], in_=ptp[:])
                    xTs.append(xT)
                pz = psum.tile([P, O], F32, name="pz", tag="pz", bufs=2)
                for k in range(kc2):
                    nc.tensor.matmul(
                        out=pz[:, :O],
                        lhsT=xTs[k][:],
                        rhs=w3_sb[:, k * O:(k + 1) * O],
                        start=(k == 0),
                        stop=(k == kc2 - 1),
                    )
                nc.scalar.copy(out=zsb[:, g * O:(g + 1) * O], in_=pz[:, :O])
                nc.sync.dma_start(out=zs[t * P:(t + 1) * P, :],
                                  in_=zsb[:, g * O:(g + 1) * O])
                if dbg_aps is not None:
                    nc.sync.dma_start(out=dbg_aps["dbg_z"][t * P:(t + 1) * P, :],
                                      in_=zsb[:, g * O:(g + 1) * O])

        nc.gpsimd.collective_compute(
            "AllGather", mybir.AluOpType.bypass, replica_groups=rg,
            ins=[zs.opt()], outs=[zf.opt()],
        )

        # ---------------- layer 3: aggregate z ----------------
        for t in range(T):
            po = psum.tile([P, O], F32, name="po", tag="pz", bufs=2)
            for c in range(K):
                ch = t * K + c
                hgz = work.tile([P, O], BF16, name="hgz", tag="hgz", bufs=8)
                nc.gpsimd.indirect_dma_start(
                    out=hgz[:],
                    out_offset=None,
                    in_=zf[:],
                    in_offset=IndirectOffsetOnAxis(
                        ap=src_sb[:, ch:ch + 1], axis=0),
                )
                mh = work.tile([P, P], BF16, name="mh", tag="mh", bufs=8)
                nc.vector.tensor_scalar(
                    out=mh[:],
                    in0=iota_bf[:],
                    scalar1=dstf_sb[:, ch:ch + 1],
                    scalar2=nrm_sb[:, ch:ch + 1],
                    op0=mybir.AluOpType.is_equal,
                    op1=mybir.AluOpType.mult,
                )
                nc.tensor.matmul(
                    out=po[:, :O],
                    lhsT=mh[:],
                    rhs=hgz[:],
                    start=(c == 0),
                    stop=False,
                )
            zown = work.tile([P, O], BF16, name="zown", tag="zown", bufs=4)
            nc.sync.dma_start(out=zown[:], in_=zs[t * P:(t + 1) * P, :])
            ms = work.tile([P, P], BF16, name="mh", tag="mh", bufs=8)
            nc.vector.tensor_scalar(
                out=ms[:],
                in0=iota_bf[:],
                scalar1=selfdst_sb[:, 0:1],
                scalar2=selfnrm_sb[:, t:t + 1],
                op0=mybir.AluOpType.is_equal,
                op1=mybir.AluOpType.mult,
            )
            nc.tensor.matmul(
                out=po[:, :O],
                lhsT=ms[:],
                rhs=zown[:],
                start=False,
                stop=True,
            )
            nc.vector.tensor_tensor(
                out=outbuf[:, t * O:(t + 1) * O],
                in0=po[:, :O],
                in1=b3row_sb[:, :O],
                op=mybir.AluOpType.add,
            )
        nc.sync.dma_start(out=out_ap, in_=outbuf[:])


# --------------------------------------------------------------------------
# execution (axon / PJRT path with device-resident timing)
# --------------------------------------------------------------------------

EXEC_NS = None


def _run_pjrt_timed(nc, in_maps, n_cores, time_iters=0):
    global EXEC_NS
    import jax
    import jax.numpy as jnp  # noqa: F401
    from jax.experimental.shard_map import shard_map
    from jax.sharding import Mesh, NamedSharding, PartitionSpec

    from concourse import bass2jax as b2j

    b2j.install_neuronx_cc_hook()

    partition_name = (nc.partition_id_tensor.name
                      if nc.partition_id_tensor else None)
    in_names, out_names, out_avals, zero_outs = [], [], [], []
    for alloc in nc.m.functions[0].allocations:
        if not isinstance(alloc, mybir.MemoryLocationSet):
            continue
        name = alloc.memorylocations[0].name
        if alloc.kind == "ExternalInput":
            if name != partition_name:
                in_names.append(name)
        elif alloc.kind == "ExternalOutput":
            out_names.append(name)
            shape = tuple(alloc.tensor_shape)
            dtype = mybir.dt.np(alloc.dtype)
            out_avals.append(jax.core.ShapedArray(shape, dtype))
            zero_outs.append(np.zeros(shape, dtype))
    n_params = len(in_names)
    all_in_names = list(in_names) + list(out_names)
    if partition_name is not None:
        all_in_names.append(partition_name)
    all_in_names = tuple(all_in_names)

    def _body(*args):
        operands = list(args)
        if partition_name is not None:
            operands.append(b2j.partition_id_tensor())
        outs = b2j._bass_exec_p.bind(
            *operands,
            out_avals=tuple(out_avals),
            in_names=all_in_names,
            out_names=tuple(out_names),
            lowering_input_output_aliases=(),
            sim_require_finite=True,
            sim_require_nnan=True,
            nc=nc,
        )
        return tuple(outs)

    devices = jax.devices()[:n_cores]
    assert len(devices) == n_cores
    mesh = Mesh(np.asarray(devices), ("core",))
    spec = PartitionSpec("core")
    n_all = n_params + len(zero_outs)
    jitted = jax.jit(shard_map(
        _body, mesh=mesh, in_specs=(spec,) * n_all,
        out_specs=(spec,) * len(out_names), check_rep=False))

    sharding = NamedSharding(mesh, spec)
    g_in = [
        jax.device_put(
            np.concatenate([np.asarray(in_maps[c][nm]) for c in range(n_cores)],
                           axis=0), sharding)
        for nm in in_names
    ]
    g_zero = [
        jax.device_put(np.concatenate([z] * n_cores, axis=0), sharding)
        for z in zero_outs
    ]

    out_arrs = jitted(*g_in, *g_zero)
    jax.block_until_ready(out_arrs)
    results = [
        {nm: np.asarray(out_arrs[i]).reshape(n_cores, *out_avals[i].shape)[c]
         for i, nm in enumerate(out_names)}
        for c in range(n_cores)
    ]

    if time_iters > 0:
        triv = jax.jit(shard_map(
            lambda a: (a + 1.0,), mesh=mesh, in_specs=(spec,),
            out_specs=(spec,), check_rep=False))
        tiny = jax.device_put(np.zeros((n_cores * 8, 8), np.float32), sharding)
        jax.block_until_ready(triv(tiny))
        walls, base = [], []
        for _ in range(time_iters):
            t0 = time.perf_counter()
            o = jitted(*g_in, *g_zero)
            jax.block_until_ready(o)
            walls.append(time.perf_counter() - t0)
            t0 = time.perf_counter()
            o = triv(tiny)
            jax.block_until_ready(o)
            base.append(time.perf_counter() - t0)
        walls = np.array(walls)
        base = np.array(base)
        diffs = walls - base
        EXEC_NS = int((np.min(walls) - np.min(base)) * 1e9)
        print(f"[timing] kernel min {np.min(walls)*1e3:.3f} "
              f"med {np.median(walls)*1e3:.3f} ms | base min "
              f"{np.min(base)*1e3:.3f} med {np.median(base)*1e3:.3f} ms | "
              f"min-diff {EXEC_NS/1e3:.0f} us  med-diff "
              f"{np.median(diffs)*1e6:.0f} us")
    return results


# --------------------------------------------------------------------------
# top level
# --------------------------------------------------------------------------

def kernel(x, edge_index, W1, b1, W2, b2, W3, b3, _trace=False, _time_iters=0, _debug=False):
    global LAST_RESULTS
    x = np.asarray(x, np.float32)
    edge_index = np.asarray(edge_index)
    n_nodes = x.shape[0]
    assert n_nodes == N_NODES and x.shape[1] == F_IN

    pre = _preprocess(edge_index, n_nodes, N_CORES, TILES_PER_CORE)
    T, K = TILES_PER_CORE, pre["K"]
    n_pad = pre["n_pad"]
    use_b1 = bool(np.any(np.asarray(b1) != 0))
    use_b2 = bool(np.any(np.asarray(b2) != 0))
    cfg = dict(n_cores=N_CORES, T=T, K=K, F1=F_IN, H=HIDDEN, O=F_OUT,
               use_b1=use_b1, use_b2=use_b2)

    import ml_dtypes
    bf = ml_dtypes.bfloat16
    x_dev = np.zeros((n_pad, F_IN), bf)
    x_dev[pre["perm"][:n_nodes]] = x

    common = dict(
        x=x_dev,
        iota=np.tile(np.arange(P, dtype=np.float32), (P, 1)).astype(bf),
        ident=np.eye(P, dtype=np.float32).astype(bf),
        w1=_pack_w(W1, HIDDEN).astype(bf),
        w2=_pack_w(W2, HIDDEN).astype(bf),
        w3=_pack_w(W3, F_OUT).astype(bf),
        b3row=np.tile(np.asarray(b3, np.float32), (P, 1)).copy(),
        selfdst=np.arange(P, dtype=np.float32).reshape(P, 1).copy(),
    )
    if use_b1 or use_b2:
        common["ones"] = np.ones((1, P), bf)
    if use_b1:
        common["b1row"] = np.asarray(b1, np.float32).reshape(1, HIDDEN).astype(bf)
    if use_b2:
        common["b2row"] = np.asarray(b2, np.float32).reshape(1, HIDDEN).astype(bf)

    in_maps = []
    for c in range(N_CORES):
        m = dict(common)
        m["src"] = pre["src"][c]
        m["dstf"] = pre["dstf"][c]
        m["nrm"] = pre["nrm"][c]
        m["selfnrm"] = pre["selfnrm"][c]
        m["xown"] = np.ascontiguousarray(x_dev[c * T * P:(c + 1) * T * P])
        in_maps.append(m)

    nc = bacc.Bacc("TRN2", target_bir_lowering=False, debug=False,
                   enable_asserts=False, num_devices=N_CORES)
    ins_aps = {}
    for name, arr in in_maps[0].items():
        ins_aps[name] = nc.dram_tensor(
            name, list(arr.shape), mybir.dt.from_np(arr.dtype),
            kind="ExternalInput").ap()
    out_t = nc.dram_tensor("out", [P, T * F_OUT], F32, kind="ExternalOutput")
    dbg_aps = None
    if _debug:
        kc1 = _ceil_div(F_IN, P)
        kc2 = _ceil_div(HIDDEN, P)
        dbg_aps = {
            "dbg_agg": nc.dram_tensor("dbg_agg", [P, kc2 * P], BF16,
                                      kind="ExternalOutput").ap(),
            "dbg_x2": nc.dram_tensor("dbg_x2", [T * P, HIDDEN], BF16,
                                     kind="ExternalOutput").ap(),
            "dbg_z": nc.dram_tensor("dbg_z", [T * P, F_OUT], BF16,
                                    kind="ExternalOutput").ap(),
        }

    with tile.TileContext(nc) as tc:
        _build_gcn(tc, ins_aps, out_t.ap(), cfg, dbg_aps)
    nc.compile()

    if axon_active():
        results = _run_pjrt_timed(nc, in_maps, N_CORES, time_iters=_time_iters)
    else:
        res = bass_utils.run_bass_kernel_spmd(
            nc, in_maps, core_ids=list(range(N_CORES)), trace=_trace)
        LAST_RESULTS = res
        results = res.results

    out_dev = np.zeros((n_pad, F_OUT), np.float32)
    for c in range(N_CORES):
        o = results[c]["out"]  # [P, T*O]
        rows = o.reshape(P, T, F_OUT).transpose(1, 0, 2).reshape(T * P, F_OUT)
        out_dev[c * T * P:(c + 1) * T * P] = rows
    out_full = out_dev[pre["perm"][:n_nodes]].copy()
    if _debug:
        return out_full, results, pre, in_maps
    return out_full


# revision 5
# speedup vs baseline: 1.3878x; 1.3666x over previous
"""3-layer GCN (PyG GCNConv semantics) on 8 Trainium2 NeuronCores — v2.

Strategy (graph/data parallel over nodes):
  - Nodes assigned to 8 cores x 196 tiles of 128 slots (serpentine by
    in-degree); edges (incl. self-loops) partitioned by destination tile,
    K chunks of 128 lanes per tile.
  - Aggregate-first per layer: per-chunk indirect-DMA gathers (128 rows
    per instruction; multi-index gathers only work for tiny tables on this
    HW), one-hot scatter matrices built on DVE (bf16), TensorE matmuls
    accumulate aggT [feature, dst] in PSUM.
  - Transform without transposes: out[dst, 384] = sum_k aggT_k^T @ W_k
    directly row-major in PSUM, ReLU on ACT -> bf16 row tiles.
  - Cross-core exchange via AllGather (8 cores = 4 devices x 2 cores;
    Shared-output dedups the pair). Layer 3 transform-first: z = H3 @ W3
    (2-wide) computed per tile during layer 2 (3 PE transposes + 3 tiny
    matmuls), so the second exchange is only ~100 KB/core; L3 aggregation
    gathers 4-byte rows and scatter-matmuls into [dst, 2] + b3.

kernel(**inputs) takes FULL unsharded inputs, returns FULL [200000, 2] f32.
"""

import time
from contextlib import ExitStack

import numpy as np

import concourse.bass as bass
import concourse.mybir as mybir
import concourse.tile as tile
from concourse import bacc
from concourse import bass_utils
from concourse._compat import axon_active
from concourse.bass import IndirectOffsetOnAxis

P = 128
F32 = mybir.dt.float32
BF16 = mybir.dt.bfloat16
I32 = mybir.dt.int32

N_NODES = 200000
F_IN = 165
HIDDEN = 384
F_OUT = 2
N_CORES = 8
TILES_PER_CORE = 196  # 8 * 196 * 128 = 200704 >= 200000
G = 7   # tiles per gather/scatter batch in layers 1-2 (196 = 28 * 7)
GZ = 49  # tiles per gather batch in layer 3

LAST_RESULTS = None


def _ceil_div(a, b):
    return (a + b - 1) // b


# --------------------------------------------------------------------------
# host-side preprocessing (topology only)
# --------------------------------------------------------------------------

def _preprocess(edge_index, n_nodes, n_cores, tiles_per_core):
    n_bins = n_cores * tiles_per_core
    n_pad = n_bins * P
    assert n_pad >= n_nodes

    row_all = np.asarray(edge_index[0], dtype=np.int64)
    col_all = np.asarray(edge_index[1], dtype=np.int64)
    deg = np.bincount(col_all, minlength=n_nodes).astype(np.float64) + 1.0
    dinv = 1.0 / np.sqrt(deg)
    nrm_all = (dinv[row_all] * dinv[col_all]).astype(np.float32)

    # node -> (bin, slot): serpentine over bins in descending-degree order
    d = np.zeros(n_pad, np.int64)
    d[:n_nodes] = deg.astype(np.int64)
    order = np.argsort(-d, kind="stable")
    rows_idx = np.arange(n_pad) // n_bins
    pos = np.arange(n_pad) % n_bins
    bins_of_rank = np.where(rows_idx % 2 == 0, pos, n_bins - 1 - pos)
    bin_of_node = np.empty(n_pad, np.int64)
    slot_of_node = np.empty(n_pad, np.int64)
    bin_of_node[order] = bins_of_rank
    slot_of_node[order] = rows_idx
    perm = bin_of_node * P + slot_of_node

    rdeg = np.bincount(col_all, minlength=n_nodes).astype(np.float64)
    binload = np.bincount(bin_of_node[:n_nodes], weights=rdeg, minlength=n_bins)
    K = int(np.ceil(binload.max() / P))

    # edge tables: chunk = K per tile, 128 lanes per chunk
    e_src_dev = perm[row_all]
    e_dst_dev = perm[col_all]
    e_bin = e_dst_dev // P
    e_slot = e_dst_dev % P
    eo = np.argsort(e_bin, kind="stable")
    e_bin_s = e_bin[eo]
    cnt = np.bincount(e_bin_s, minlength=n_bins)
    starts = np.concatenate([[0], np.cumsum(cnt)[:-1]])
    within = np.arange(len(e_bin_s)) - starts[e_bin_s]
    lane = within % P
    chunk_global = e_bin_s * K + within // P

    n_ch = n_bins * K
    src_t = np.zeros((n_ch, P), np.int32)
    dst_t = np.zeros((n_ch, P), np.float32)
    nrm_t = np.zeros((n_ch, P), np.float32)
    src_t[chunk_global, lane] = e_src_dev[eo].astype(np.int32)
    dst_t[chunk_global, lane] = e_slot[eo].astype(np.float32)
    nrm_t[chunk_global, lane] = nrm_all[eo]

    TK = tiles_per_core * K
    src = src_t.reshape(n_cores, TK, P).transpose(0, 2, 1).copy()
    dstf = dst_t.reshape(n_cores, TK, P).transpose(0, 2, 1).copy()
    nrm = nrm_t.reshape(n_cores, TK, P).transpose(0, 2, 1).copy()

    node_of_row = np.full(n_pad, -1, np.int64)
    node_of_row[perm[:n_nodes]] = np.arange(n_nodes)
    dinv2_row = np.zeros(n_pad, np.float32)
    ok = node_of_row >= 0
    dinv2_row[ok] = (dinv[node_of_row[ok]] ** 2).astype(np.float32)
    selfnrm = dinv2_row.reshape(n_cores, tiles_per_core, P).transpose(0, 2, 1).copy()
    return dict(perm=perm, K=K, src=src, dstf=dstf, nrm=nrm, n_pad=n_pad,
                selfnrm=selfnrm)


def _pack_w(W, f_out):
    f_in = W.shape[0]
    kc = _ceil_div(f_in, P)
    Wp = np.zeros((kc * P, f_out), np.float32)
    Wp[:f_in] = np.asarray(W, np.float32)
    return Wp.reshape(kc, P, f_out).transpose(1, 0, 2).reshape(P, kc * f_out).copy()


# --------------------------------------------------------------------------
# device program
# --------------------------------------------------------------------------

def _build_gcn(tc, ins, out_ap, cfg, dbg_aps=None):
    nc = tc.nc
    n_cores = cfg["n_cores"]
    T, K = cfg["T"], cfg["K"]
    F1, H, O = cfg["F1"], cfg["H"], cfg["O"]
    use_b1, use_b2 = cfg["use_b1"], cfg["use_b2"]
    n_pad = n_cores * T * P
    kc1 = _ceil_div(F1, P)
    kc2 = _ceil_div(H, P)
    rg = [list(range(n_cores))]

    ctx = ExitStack()
    with ctx:
        const = ctx.enter_context(tc.tile_pool(name="const", bufs=1))
        dram = ctx.enter_context(tc.tile_pool(name="dram", bufs=1, space="DRAM"))
        work = ctx.enter_context(tc.tile_pool(name="work", bufs=2))
        psum = ctx.enter_context(tc.tile_pool(name="psum", bufs=2, space="PSUM"))

        def load_const(name, shape, dtype=F32):
            t = const.tile(list(shape), dtype, name=name)
            nc.sync.dma_start(out=t[:], in_=ins[name][:])
            return t

        iota_bf = load_const("iota", [P, P], BF16)
        ident_bf = load_const("ident", [P, P], BF16)
        w1_sb = load_const("w1", [P, kc1 * H], BF16)
        w2_sb = load_const("w2", [P, kc2 * H], BF16)
        w3_sb = load_const("w3", [P, kc2 * O], BF16)
        b3row_sb = load_const("b3row", [P, O])
        src_sb = load_const("src", [P, T * K], I32)
        dstf_sb = load_const("dstf", [P, T * K])
        nrm_sb = load_const("nrm", [P, T * K])
        selfdst_sb = load_const("selfdst", [P, 1])
        selfnrm_sb = load_const("selfnrm", [P, T])
        if use_b1 or use_b2:
            ones_sb = load_const("ones", [1, P], BF16)
        if use_b1:
            b1row_sb = load_const("b1row", [1, H], BF16)
        if use_b2:
            b2row_sb = load_const("b2row", [1, H], BF16)

        outbuf = const.tile([P, T * O], F32, name="outbuf")

        x2s = dram.tile([T * P, H], BF16, name="x2s")
        x2f = dram.tile([n_pad, H], BF16, name="x2f", addr_space="Shared")
        zs = dram.tile([T * P, O], BF16, name="zs")
        zf = dram.tile([n_pad, O], BF16, name="zf", addr_space="Shared")

        def agg_tile(x_src_ap, self_src_ap, F, kc, t):
            """aggT[f, dst] = sum_c gathered_c[:, f]^T @ onehot_c
            plus the self-loop chunk loaded with a plain contiguous DMA"""
            hgs, mhs = [], []
            for c in range(K):
                ch = t * K + c
                hg = work.tile([P, H], BF16, name="hg", tag="hg", bufs=16)
                gi = nc.gpsimd.indirect_dma_start(
                    out=hg[:, :F],
                    out_offset=None,
                    in_=x_src_ap,
                    in_offset=IndirectOffsetOnAxis(
                        ap=src_sb[:, ch:ch + 1], axis=0),
                )
                q = ch % 4
                if q:
                    gi.ins.queue = f"qPoolDynamic{q}"
                mh = work.tile([P, P], BF16, name="mh", tag="mh", bufs=16)
                nc.vector.tensor_scalar(
                    out=mh[:],
                    in0=iota_bf[:],
                    scalar1=dstf_sb[:, ch:ch + 1],
                    scalar2=nrm_sb[:, ch:ch + 1],
                    op0=mybir.AluOpType.is_equal,
                    op1=mybir.AluOpType.mult,
                )
                hgs.append(hg)
                mhs.append(mh)
            hs = work.tile([P, H], BF16, name="hg", tag="hg", bufs=16)
            nc.sync.dma_start(out=hs[:, :F],
                              in_=self_src_ap[t * P:(t + 1) * P, :])
            ms = work.tile([P, P], BF16, name="mh", tag="mh", bufs=16)
            nc.vector.tensor_scalar(
                out=ms[:],
                in0=iota_bf[:],
                scalar1=selfdst_sb[:, 0:1],
                scalar2=selfnrm_sb[:, t:t + 1],
                op0=mybir.AluOpType.is_equal,
                op1=mybir.AluOpType.mult,
            )
            hgs.append(hs)
            mhs.append(ms)
            aggT = work.tile([P, kc2 * P], BF16, name="aggT", tag="aggT", bufs=4)
            for f in range(kc):
                fw = min(P, F - f * P)
                pa = psum.tile([P, P], F32, name="pa", tag="pa", bufs=2)
                for c in range(K + 1):
                    nc.tensor.matmul(
                        out=pa[:fw, :],
                        lhsT=hgs[c][:, f * P:f * P + fw],
                        rhs=mhs[c][:],
                        start=(c == 0),
                        stop=(c == K),
                    )
                nc.scalar.copy(out=aggT[:fw, f * P:(f + 1) * P], in_=pa[:fw, :])
            return aggT

        def transform_tile(aggT, kc_in, F_in, w_sb, brow):
            """pt[dst, j] = sum_k aggT_k^T @ W_k (+ ones^T @ brow)"""
            pt = psum.tile([P, H], F32, name="pt", tag="pt", bufs=2)
            for k in range(kc_in):
                kw = min(P, F_in - k * P)
                nc.tensor.matmul(
                    out=pt[:, :H],
                    lhsT=aggT[:kw, k * P:k * P + P],
                    rhs=w_sb[:kw, k * H:k * H + H],
                    start=(k == 0),
                    stop=(k == kc_in - 1 and brow is None),
                )
            if brow is not None:
                nc.tensor.matmul(
                    out=pt[:, :H],
                    lhsT=ones_sb[:1, :P],
                    rhs=brow[:1, :H],
                    start=False,
                    stop=True,
                )
            return pt

        nb = T // G

        # ---------------- layer 1 ----------------
        for b in range(nb):
            xsb = work.tile([P, G * H], BF16, name="xsb", tag="xsb", bufs=2)
            for g in range(G):
                t = b * G + g
                aggT = agg_tile(ins["x"][:], ins["xown"], F1, kc1, t)
                if dbg_aps is not None and t == 0:
                    nc.sync.dma_start(out=dbg_aps["dbg_agg"], in_=aggT[:])
                pt = transform_tile(aggT, kc1, F1, w1_sb,
                                    b1row_sb if use_b1 else None)
                nc.scalar.activation(
                    out=xsb[:, g * H:(g + 1) * H],
                    in_=pt[:, :H],
                    func=mybir.ActivationFunctionType.Relu,
                )
                nc.sync.dma_start(out=x2s[t * P:(t + 1) * P, :],
                                  in_=xsb[:, g * H:(g + 1) * H])
                if dbg_aps is not None:
                    nc.sync.dma_start(out=dbg_aps["dbg_x2"][t * P:(t + 1) * P, :],
                                      in_=xsb[:, g * H:(g + 1) * H])

        nc.gpsimd.collective_compute(
            "AllGather", mybir.AluOpType.bypass, replica_groups=rg,
            ins=[x2s.opt()], outs=[x2f.opt()],
        )

        # ---------------- layer 2 (+ z = relu-out @ W3) ----------------
        for b in range(nb):
            zsb = work.tile([P, G * O], BF16, name="zsb", tag="zsb", bufs=2)
            for g in range(G):
                t = b * G + g
                aggT = agg_tile(x2f[:], x2s, H, kc2, t)
                pt = transform_tile(aggT, kc2, H, w2_sb,
                                    b2row_sb if use_b2 else None)
                x3row = work.tile([P, H], BF16, name="x3r", tag="x3r", bufs=3)
                nc.scalar.activation(
                    out=x3row[:],
                    in_=pt[:, :H],
                    func=mybir.ActivationFunctionType.Relu,
                )
                xTs = []
                for k in range(kc2):
                    ptp = psum.tile([P, P], BF16, name="ptp", tag="ptp", bufs=2)
                    nc.tensor.transpose(
                        out=ptp[:],
                        in_=x3row[:, k * P:(k + 1) * P],
                        identity=ident_bf[:],
                    )
                    xT = work.tile([P, P], BF16, name="xT", tag="xT", bufs=4)
                    nc.vector.tensor_copy(out=xT[:], in_=ptp[:])
                    xTs.append(xT)
                pz = psum.tile([P, O], F32, name="pz", tag="pz", bufs=2)
                for k in range(kc2):
                    nc.tensor.matmul(
                        out=pz[:, :O],
                        lhsT=xTs[k][:],
                        rhs=w3_sb[:, k * O:(k + 1) * O],
                        start=(k == 0),
                        stop=(k == kc2 - 1),
                    )
                nc.scalar.copy(out=zsb[:, g * O:(g + 1) * O], in_=pz[:, :O])
                nc.sync.dma_start(out=zs[t * P:(t + 1) * P, :],
                                  in_=zsb[:, g * O:(g + 1) * O])
                if dbg_aps is not None:
                    nc.sync.dma_start(out=dbg_aps["dbg_z"][t * P:(t + 1) * P, :],
                                      in_=zsb[:, g * O:(g + 1) * O])

        nc.gpsimd.collective_compute(
            "AllGather", mybir.AluOpType.bypass, replica_groups=rg,
            ins=[zs.opt()], outs=[zf.opt()],
        )

        # ---------------- layer 3: aggregate z ----------------
        for t in range(T):
            po = psum.tile([P, O], F32, name="po", tag="pz", bufs=2)
            for c in range(K):
                ch = t * K + c
                hgz = work.tile([P, O], BF16, name="hgz", tag="hgz", bufs=16)
                gi3 = nc.gpsimd.indirect_dma_start(
                    out=hgz[:],
                    out_offset=None,
                    in_=zf[:],
                    in_offset=IndirectOffsetOnAxis(
                        ap=src_sb[:, ch:ch + 1], axis=0),
                )
                q3 = ch % 4
                if q3:
                    gi3.ins.queue = f"qPoolDynamic{q3}"
                mh = work.tile([P, P], BF16, name="mh", tag="mh", bufs=16)
                nc.vector.tensor_scalar(
                    out=mh[:],
                    in0=iota_bf[:],
                    scalar1=dstf_sb[:, ch:ch + 1],
                    scalar2=nrm_sb[:, ch:ch + 1],
                    op0=mybir.AluOpType.is_equal,
                    op1=mybir.AluOpType.mult,
                )
                nc.tensor.matmul(
                    out=po[:, :O],
                    lhsT=mh[:],
                    rhs=hgz[:],
                    start=(c == 0),
                    stop=False,
                )
            zown = work.tile([P, O], BF16, name="zown", tag="zown", bufs=4)
            nc.sync.dma_start(out=zown[:], in_=zs[t * P:(t + 1) * P, :])
            ms = work.tile([P, P], BF16, name="mh", tag="mh", bufs=16)
            nc.vector.tensor_scalar(
                out=ms[:],
                in0=iota_bf[:],
                scalar1=selfdst_sb[:, 0:1],
                scalar2=selfnrm_sb[:, t:t + 1],
                op0=mybir.AluOpType.is_equal,
                op1=mybir.AluOpType.mult,
            )
            nc.tensor.matmul(
                out=po[:, :O],
                lhsT=ms[:],
                rhs=zown[:],
                start=False,
                stop=True,
            )
            nc.vector.tensor_tensor(
                out=outbuf[:, t * O:(t + 1) * O],
                in0=po[:, :O],
                in1=b3row_sb[:, :O],
                op=mybir.AluOpType.add,
            )
        nc.sync.dma_start(out=out_ap, in_=outbuf[:])


# --------------------------------------------------------------------------
# execution (axon / PJRT path with device-resident timing)
# --------------------------------------------------------------------------

EXEC_NS = None


def _run_pjrt_timed(nc, in_maps, n_cores, time_iters=0):
    global EXEC_NS
    import jax
    import jax.numpy as jnp  # noqa: F401
    from jax.experimental.shard_map import shard_map
    from jax.sharding import Mesh, NamedSharding, PartitionSpec

    from concourse import bass2jax as b2j

    b2j.install_neuronx_cc_hook()

    partition_name = (nc.partition_id_tensor.name
                      if nc.partition_id_tensor else None)
    in_names, out_names, out_avals, zero_outs = [], [], [], []
    for alloc in nc.m.functions[0].allocations:
        if not isinstance(alloc, mybir.MemoryLocationSet):
            continue
        name = alloc.memorylocations[0].name
        if alloc.kind == "ExternalInput":
            if name != partition_name:
                in_names.append(name)
        elif alloc.kind == "ExternalOutput":
            out_names.append(name)
            shape = tuple(alloc.tensor_shape)
            dtype = mybir.dt.np(alloc.dtype)
            out_avals.append(jax.core.ShapedArray(shape, dtype))
            zero_outs.append(np.zeros(shape, dtype))
    n_params = len(in_names)
    all_in_names = list(in_names) + list(out_names)
    if partition_name is not None:
        all_in_names.append(partition_name)
    all_in_names = tuple(all_in_names)

    def _body(*args):
        operands = list(args)
        if partition_name is not None:
            operands.append(b2j.partition_id_tensor())
        outs = b2j._bass_exec_p.bind(
            *operands,
            out_avals=tuple(out_avals),
            in_names=all_in_names,
            out_names=tuple(out_names),
            lowering_input_output_aliases=(),
            sim_require_finite=True,
            sim_require_nnan=True,
            nc=nc,
        )
        return tuple(outs)

    devices = jax.devices()[:n_cores]
    assert len(devices) == n_cores
    mesh = Mesh(np.asarray(devices), ("core",))
    spec = PartitionSpec("core")
    n_all = n_params + len(zero_outs)
    jitted = jax.jit(shard_map(
        _body, mesh=mesh, in_specs=(spec,) * n_all,
        out_specs=(spec,) * len(out_names), check_rep=False))

    sharding = NamedSharding(mesh, spec)
    g_in = [
        jax.device_put(
            np.concatenate([np.asarray(in_maps[c][nm]) for c in range(n_cores)],
                           axis=0), sharding)
        for nm in in_names
    ]
    g_zero = [
        jax.device_put(np.concatenate([z] * n_cores, axis=0), sharding)
        for z in zero_outs
    ]

    out_arrs = jitted(*g_in, *g_zero)
    jax.block_until_ready(out_arrs)
    results = [
        {nm: np.asarray(out_arrs[i]).reshape(n_cores, *out_avals[i].shape)[c]
         for i, nm in enumerate(out_names)}
        for c in range(n_cores)
    ]

    if time_iters > 0:
        triv = jax.jit(shard_map(
            lambda a: (a + 1.0,), mesh=mesh, in_specs=(spec,),
            out_specs=(spec,), check_rep=False))
        tiny = jax.device_put(np.zeros((n_cores * 8, 8), np.float32), sharding)
        jax.block_until_ready(triv(tiny))
        walls, base = [], []
        for _ in range(time_iters):
            t0 = time.perf_counter()
            o = jitted(*g_in, *g_zero)
            jax.block_until_ready(o)
            walls.append(time.perf_counter() - t0)
            t0 = time.perf_counter()
            o = triv(tiny)
            jax.block_until_ready(o)
            base.append(time.perf_counter() - t0)
        walls = np.array(walls)
        base = np.array(base)
        diffs = walls - base
        EXEC_NS = int((np.min(walls) - np.min(base)) * 1e9)
        print(f"[timing] kernel min {np.min(walls)*1e3:.3f} "
              f"med {np.median(walls)*1e3:.3f} ms | base min "
              f"{np.min(base)*1e3:.3f} med {np.median(base)*1e3:.3f} ms | "
              f"min-diff {EXEC_NS/1e3:.0f} us  med-diff "
              f"{np.median(diffs)*1e6:.0f} us")
    return results


# --------------------------------------------------------------------------
# top level
# --------------------------------------------------------------------------

def kernel(x, edge_index, W1, b1, W2, b2, W3, b3, _trace=False, _time_iters=0, _debug=False):
    global LAST_RESULTS
    x = np.asarray(x, np.float32)
    edge_index = np.asarray(edge_index)
    n_nodes = x.shape[0]
    assert n_nodes == N_NODES and x.shape[1] == F_IN

    pre = _preprocess(edge_index, n_nodes, N_CORES, TILES_PER_CORE)
    T, K = TILES_PER_CORE, pre["K"]
    n_pad = pre["n_pad"]
    use_b1 = bool(np.any(np.asarray(b1) != 0))
    use_b2 = bool(np.any(np.asarray(b2) != 0))
    cfg = dict(n_cores=N_CORES, T=T, K=K, F1=F_IN, H=HIDDEN, O=F_OUT,
               use_b1=use_b1, use_b2=use_b2)

    import ml_dtypes
    bf = ml_dtypes.bfloat16
    x_dev = np.zeros((n_pad, F_IN), bf)
    x_dev[pre["perm"][:n_nodes]] = x

    common = dict(
        x=x_dev,
        iota=np.tile(np.arange(P, dtype=np.float32), (P, 1)).astype(bf),
        ident=np.eye(P, dtype=np.float32).astype(bf),
        w1=_pack_w(W1, HIDDEN).astype(bf),
        w2=_pack_w(W2, HIDDEN).astype(bf),
        w3=_pack_w(W3, F_OUT).astype(bf),
        b3row=np.tile(np.asarray(b3, np.float32), (P, 1)).copy(),
        selfdst=np.arange(P, dtype=np.float32).reshape(P, 1).copy(),
    )
    if use_b1 or use_b2:
        common["ones"] = np.ones((1, P), bf)
    if use_b1:
        common["b1row"] = np.asarray(b1, np.float32).reshape(1, HIDDEN).astype(bf)
    if use_b2:
        common["b2row"] = np.asarray(b2, np.float32).reshape(1, HIDDEN).astype(bf)

    in_maps = []
    for c in range(N_CORES):
        m = dict(common)
        m["src"] = pre["src"][c]
        m["dstf"] = pre["dstf"][c]
        m["nrm"] = pre["nrm"][c]
        m["selfnrm"] = pre["selfnrm"][c]
        m["xown"] = np.ascontiguousarray(x_dev[c * T * P:(c + 1) * T * P])
        in_maps.append(m)

    nc = bacc.Bacc("TRN2", target_bir_lowering=False, debug=False,
                   enable_asserts=False, num_devices=N_CORES,
                   num_swdge_queues=4)
    ins_aps = {}
    for name, arr in in_maps[0].items():
        ins_aps[name] = nc.dram_tensor(
            name, list(arr.shape), mybir.dt.from_np(arr.dtype),
            kind="ExternalInput").ap()
    out_t = nc.dram_tensor("out", [P, T * F_OUT], F32, kind="ExternalOutput")
    dbg_aps = None
    if _debug:
        kc1 = _ceil_div(F_IN, P)
        kc2 = _ceil_div(HIDDEN, P)
        dbg_aps = {
            "dbg_agg": nc.dram_tensor("dbg_agg", [P, kc2 * P], BF16,
                                      kind="ExternalOutput").ap(),
            "dbg_x2": nc.dram_tensor("dbg_x2", [T * P, HIDDEN], BF16,
                                     kind="ExternalOutput").ap(),
            "dbg_z": nc.dram_tensor("dbg_z", [T * P, F_OUT], BF16,
                                    kind="ExternalOutput").ap(),
        }

    with tile.TileContext(nc) as tc:
        _build_gcn(tc, ins_aps, out_t.ap(), cfg, dbg_aps)
    nc.compile()

    if axon_active():
        results = _run_pjrt_timed(nc, in_maps, N_CORES, time_iters=_time_iters)
    else:
        res = bass_utils.run_bass_kernel_spmd(
            nc, in_maps, core_ids=list(range(N_CORES)), trace=_trace)
        LAST_RESULTS = res
        results = res.results

    out_dev = np.zeros((n_pad, F_OUT), np.float32)
    for c in range(N_CORES):
        o = results[c]["out"]  # [P, T*O]
        rows = o.reshape(P, T, F_OUT).transpose(1, 0, 2).reshape(T * P, F_OUT)
        out_dev[c * T * P:(c + 1) * T * P] = rows
    out_full = out_dev[pre["perm"][:n_nodes]].copy()
    if _debug:
        return out_full, results, pre, in_maps
    return out_full
